# revision 7
# baseline (speedup 1.0000x reference)
"""Trainium2 Bass kernel for nn_Decoder_17214228922493.

32-step LSTM decoder: B=64, H=1536, input=1024, applied to a constant input.
    xg = x @ W_ih.T + b_ih + b_hh                      (once)
    per step: gates = xg + h @ W_hh.T ; LSTM cell update ; emit h

Sharding: tensor-parallel over the gate dimension (8 cores x 768 gate
columns); after every step the 8 h^T slices are re-assembled with an
AllGather.  Gate columns are reordered per core to [i | f | o | g].

Batch rows are independent, so the 64-row batch is split into two 32-row
chains that interleave on the engines: while chain A's AllGather is in
flight, chain B's matmuls/eltwise run (keeps TensorE HAM-warm and hides
the collective latency).

Matmul operands are bf16 (fp32 matmul costs 2 PE passes); PSUM
accumulation and the cell-state arithmetic stay fp32.
"""

import sys

if "/opt/trn_rl_repo" not in sys.path:
    sys.path.insert(0, "/opt/trn_rl_repo")

from contextlib import ExitStack

import ml_dtypes
import numpy as np

import concourse.bass as bass
import concourse.mybir as mybir
import concourse.tile as tile
from concourse import bacc
from concourse import bass_utils
from concourse._compat import get_trn_type

F32 = mybir.dt.float32
BF16 = mybir.dt.bfloat16
R = 8          # cores
B = 64         # batch
BC = 32        # batch rows per chain
H = 1536       # hidden
HL = H // R    # 192 per-core hidden slice
IN = 1024      # lstm input size
KA = 1152      # augmented input contraction (1024 + bias row, padded to 9*128)
NG = 4 * HL    # 768 gate columns per core
S = 32         # steps
NH = 384       # matmul moving free-dim (two groups of 384 = NG)
KHT = H // 128   # 12 k-tiles for the recurrent matmul
KAT = KA // 128  # 9 k-tiles for the input matmul

_CACHE = {}


def _build():
    nc = bacc.Bacc(
        get_trn_type() or "TRN2",
        target_bir_lowering=False,
        debug=False,
        num_devices=R,
    )

    xT = nc.dram_tensor("xT", [KA, B], BF16, kind="ExternalInput")
    wih = nc.dram_tensor("wih", [KA, NG], BF16, kind="ExternalInput")
    whh = nc.dram_tensor("whh", [H, NG], BF16, kind="ExternalInput")
    h0T = nc.dram_tensor("h0T", [H, B], BF16, kind="ExternalInput")
    c0 = nc.dram_tensor("c0", [B, HL], F32, kind="ExternalInput")
    iden = nc.dram_tensor("iden", [B, B], BF16, kind="ExternalInput")
    out = nc.dram_tensor("out", [S, HL, B], BF16, kind="ExternalOutput")

    bounces = [
        [
            nc.dram_tensor(f"bounce{c}_{t}", [HL, BC], BF16, kind="Internal")
            for t in range(S)
        ]
        for c in range(2)
    ]
    gaths = [
        [
            nc.dram_tensor(
                f"gath{c}_{t}", [H, BC], BF16, kind="Internal", addr_space="Shared"
            )
            for t in range(S - 1)
        ]
        for c in range(2)
    ]

    sig = mybir.ActivationFunctionType.Sigmoid
    tanh = mybir.ActivationFunctionType.Tanh

    with ExitStack() as ctx:
        tc = ctx.enter_context(tile.TileContext(nc))
        wpool = ctx.enter_context(tc.tile_pool(name="w", bufs=1))
        cpool = ctx.enter_context(tc.tile_pool(name="cst", bufs=1))
        hpool = ctx.enter_context(tc.tile_pool(name="h", bufs=2))
        spool = ctx.enter_context(tc.tile_pool(name="s", bufs=3))
        gpool = ctx.enter_context(tc.tile_pool(name="g", bufs=2, space="PSUM"))
        tpool = ctx.enter_context(tc.tile_pool(name="t", bufs=2, space="PSUM"))

        whh_t = []
        for k in range(KHT):
            w = wpool.tile([128, NG], BF16, tag=f"whh{k}")
            nc.sync.dma_start(w[:], whh[128 * k : 128 * (k + 1), :])
            whh_t.append(w)
        wih_t = []
        for k in range(KAT):
            w = wpool.tile([128, NG], BF16, tag=f"wih{k}")
            nc.sync.dma_start(w[:], wih[128 * k : 128 * (k + 1), :])
            wih_t.append(w)
        x_t = []
        for k in range(KAT):
            xx = wpool.tile([128, B], BF16, tag=f"x{k}")
            nc.sync.dma_start(xx[:], xT[128 * k : 128 * (k + 1), :])
            x_t.append(xx)
        iden_t = cpool.tile([B, B], BF16, tag="iden")
        nc.sync.dma_start(iden_t[:], iden[:])

        # chain state: h^T chunk tiles [128, 4*BC] x3 per chain, c [BC, HL]
        def load_hT(src, c, cols):
            chunks = []
            for j in range(3):
                hc = hpool.tile([128, 4 * BC], BF16, tag=f"hc{c}_{j}")
                src_ap = src.rearrange("(k p) n -> p k n", p=128)[
                    :, 4 * j : 4 * j + 4, cols
                ]
                nc.sync.dma_start(hc[:], src_ap)
                chunks.append(hc)
            return chunks

        def h_tile(chunks, k):
            return chunks[k // 4][:, BC * (k % 4) : BC * (k % 4 + 1)]

        h_chunks = [
            load_hT(h0T, 0, slice(0, BC)),
            load_hT(h0T, 1, slice(BC, B)),
        ]
        c_t = []
        for c in range(2):
            ct = spool.tile([BC, HL], F32, tag=f"c{c}")
            nc.sync.dma_start(ct[:], c0[BC * c : BC * (c + 1), :])
            c_t.append(ct)

        # xg = xT.T @ wih once for the full batch (bias folded in row 1024)
        xg_ps = []
        for n in range(2):
            p = gpool.tile([B, NH], F32, tag=f"g{n}")
            for k in range(KAT):
                nc.tensor.matmul(
                    p[:],
                    x_t[k][:],
                    wih_t[k][:, bass.ts(n, NH)],
                    start=(k == 0),
                    stop=(k == KAT - 1),
                )
            xg_ps.append(p)
        xg_sb = cpool.tile([B, NG], BF16, tag="xg")
        nc.vector.tensor_copy(xg_sb[:, 0:NH], xg_ps[0][:])
        nc.vector.tensor_copy(xg_sb[:, NH:NG], xg_ps[1][:])

        def chain_step(c, t):
            rows = slice(BC * c, BC * (c + 1))
            # gates = xg + h @ whh, two 1-bank PSUM groups [i|f] and [o|g]
            halves = []
            for n in range(2):
                nsl = bass.ts(n, NH)
                p = gpool.tile([BC, NH], F32, tag=f"g{n}")
                nc.tensor.matmul(
                    p[:],
                    iden_t[rows, rows],
                    xg_sb[rows, nsl],
                    start=True,
                    stop=False,
                )
                for k in range(KHT):
                    nc.tensor.matmul(
                        p[:],
                        h_tile(h_chunks[c], k),
                        whh_t[k][:, nsl],
                        start=False,
                        stop=(k == KHT - 1),
                    )
                halves.append(p)
            psA, psB = halves

            s_if = spool.tile([BC, 2 * HL], F32, tag="sif")
            nc.scalar.activation(s_if[:], psA[:], sig)
            s_o = spool.tile([BC, HL], F32, tag="so")
            nc.scalar.activation(s_o[:], psB[:, 0:HL], sig)
            tg = spool.tile([BC, HL], F32, tag="tg")
            nc.scalar.activation(tg[:], psB[:, HL : 2 * HL], tanh)

            m1 = spool.tile([BC, HL], F32, tag="m1")
            nc.vector.tensor_mul(m1[:], s_if[:, HL : 2 * HL], c_t[c][:])  # f*c
            m2 = spool.tile([BC, HL], F32, tag="m2")
            nc.vector.tensor_mul(m2[:], s_if[:, 0:HL], tg[:])  # i*g
            c_new = spool.tile([BC, HL], F32, tag=f"c{c}")
            nc.vector.tensor_add(c_new[:], m1[:], m2[:])
            c_t[c] = c_new
            tc_sb = spool.tile([BC, HL], F32, tag="tc")
            nc.scalar.activation(tc_sb[:], c_new[:], tanh)
            h_sb = spool.tile([BC, HL], BF16, tag="hsb")
            nc.vector.tensor_mul(h_sb[:], s_o[:], tc_sb[:])

            # transpose h [BC, 192] -> [192, BC] via PE (two blocks in one bank)
            tp = tpool.tile([128, 2 * BC], BF16, tag="ht")
            nc.tensor.transpose(tp[:, 0:BC], h_sb[:, 0:128], iden_t[0:BC, 0:BC])
            nc.tensor.transpose(
                tp[0:64, BC : 2 * BC], h_sb[:, 128:HL], iden_t[0:BC, 0:BC]
            )
            ht0 = spool.tile([128, BC], BF16, tag="ht0s")
            nc.vector.tensor_copy(ht0[:], tp[:, 0:BC])
            ht1 = spool.tile([64, BC], BF16, tag="ht1s")
            nc.vector.tensor_copy(ht1[:], tp[0:64, BC : 2 * BC])

            nc.sync.dma_start(bounces[c][t][0:128, :], ht0[:])
            nc.sync.dma_start(bounces[c][t][128:HL, :], ht1[:])
            # final output written from the bounce copy, off the critical path
            nc.gpsimd.dma_start(out[t, :, rows], bounces[c][t][:])

            if t < S - 1:
                nc.gpsimd.collective_compute(
                    "AllGather",
                    mybir.AluOpType.bypass,
                    replica_groups=[list(range(R))],
                    ins=[bounces[c][t][:]],
                    outs=[gaths[c][t][:]],
                )
                h_chunks[c] = load_hT(gaths[c][t], c, slice(0, BC))

        for t in range(S):
            chain_step(0, t)
            chain_step(1, t)

    nc.compile()
    return nc


def _prep_inputs(sequence, hidden_state, cell_state, W_ih, W_hh, b_ih, b_hh):
    x = np.asarray(sequence, np.float32)[0]          # [64, 1024]
    h0 = np.asarray(hidden_state, np.float32)[0]     # [64, 1536]
    c0f = np.asarray(cell_state, np.float32)[0]
    W_ih = np.asarray(W_ih, np.float32)
    W_hh = np.asarray(W_hh, np.float32)
    b = (np.asarray(b_ih, np.float32) + np.asarray(b_hh, np.float32))

    bf = ml_dtypes.bfloat16
    xT = np.zeros((KA, B), np.float32)
    xT[:IN] = x.T
    xT[IN] = 1.0
    xT = xT.astype(bf)
    h0T = np.ascontiguousarray(h0.T).astype(bf)
    iden = np.eye(B, dtype=bf)

    in_maps = []
    for r in range(R):
        sl = np.arange(r * HL, (r + 1) * HL)
        sel = np.concatenate([sl, H + sl, 3 * H + sl, 2 * H + sl])  # i, f, o, g
        wa = np.zeros((KA, NG), np.float32)
        wa[:IN] = W_ih[sel].T
        wa[IN] = b[sel]
        in_maps.append(
            {
                "xT": xT,
                "wih": wa.astype(bf),
                "whh": np.ascontiguousarray(W_hh[sel].T).astype(bf),
                "h0T": h0T,
                "c0": np.ascontiguousarray(c0f[:, sl]),
                "iden": iden,
            }
        )
    return in_maps


def kernel(**inputs) -> np.ndarray:
    if "nc" not in _CACHE:
        _CACHE["nc"] = _build()
    nc = _CACHE["nc"]
    in_maps = _prep_inputs(**inputs)
    res = bass_utils.run_bass_kernel_spmd(nc, in_maps, core_ids=list(range(R)))
    preds = np.empty((S, B, H), np.float32)
    for r in range(R):
        o = np.asarray(res.results[r]["out"], np.float32)  # [32, 192, 64]
        preds[:, :, r * HL : (r + 1) * HL] = np.transpose(o, (0, 2, 1))
    return preds


# revision 8
# speedup vs baseline: 1.0759x; 1.0759x over previous
"""Trainium2 Bass kernel for nn_Decoder_17214228922493.

32-step LSTM decoder: B=64, H=1536, input=1024, applied to a constant input.
    xg = x @ W_ih.T + b_ih + b_hh                      (once per step, see below)
    per step: gates = xg + h @ W_hh.T ; LSTM cell update ; emit h

Sharding: tensor-parallel over the gate dimension (8 cores x 768 gate
columns); after every step the 8 h^T slices are re-assembled with an
AllGather (mesh, ~6us).  Gate columns are reordered per core to
[f | o | i | g] so one sigmoid covers f,o (whose products with c can start
while the second matmul group is still streaming).

The xg contribution is re-computed from x every step instead of being
injected from a saved tile: those 18 matmuls have no dependency on h, so
the scheduler runs them inside the AllGather window — free work that also
keeps the PE HAM-warm (idle >3.4us would re-throttle the clock 2.4->1.2GHz).

Matmul operands are bf16 (fp32 matmul costs 2 PE passes); PSUM
accumulation and the cell-state arithmetic stay fp32.
"""

import sys

if "/opt/trn_rl_repo" not in sys.path:
    sys.path.insert(0, "/opt/trn_rl_repo")

from contextlib import ExitStack

import ml_dtypes
import numpy as np

import concourse.bass as bass
import concourse.mybir as mybir
import concourse.tile as tile
from concourse import bacc
from concourse import bass_utils
from concourse._compat import get_trn_type

F32 = mybir.dt.float32
BF16 = mybir.dt.bfloat16
R = 8          # cores
B = 64         # batch
H = 1536       # hidden
HL = H // R    # 192 per-core hidden slice
IN = 1024      # lstm input size
KA = 1152      # augmented input contraction (1024 + bias row, padded to 9*128)
NG = 4 * HL    # 768 gate columns per core
S = 32         # steps
NH = 384       # matmul moving free-dim (two groups of 384 = NG)
KHT = H // 128   # 12 k-tiles for the recurrent matmul
KAT = KA // 128  # 9 k-tiles for the input matmul

_CACHE = {}


def _build():
    nc = bacc.Bacc(
        get_trn_type() or "TRN2",
        target_bir_lowering=False,
        debug=False,
        num_devices=R,
    )

    xT = nc.dram_tensor("xT", [KA, B], BF16, kind="ExternalInput")
    wih = nc.dram_tensor("wih", [KA, NG], BF16, kind="ExternalInput")
    whh = nc.dram_tensor("whh", [H, NG], BF16, kind="ExternalInput")
    h0T = nc.dram_tensor("h0T", [H, B], BF16, kind="ExternalInput")
    c0 = nc.dram_tensor("c0", [B, HL], F32, kind="ExternalInput")
    iden = nc.dram_tensor("iden", [B, B], BF16, kind="ExternalInput")
    out = nc.dram_tensor("out", [S, HL, B], BF16, kind="ExternalOutput")

    bounces = [
        nc.dram_tensor(f"bounce{t}", [HL, B], BF16, kind="Internal") for t in range(S)
    ]
    gaths = [
        nc.dram_tensor(f"gath{t}", [H, B], BF16, kind="Internal", addr_space="Shared")
        for t in range(S - 1)
    ]

    sig = mybir.ActivationFunctionType.Sigmoid
    tanh = mybir.ActivationFunctionType.Tanh

    with ExitStack() as ctx:
        tc = ctx.enter_context(tile.TileContext(nc))
        wpool = ctx.enter_context(tc.tile_pool(name="w", bufs=1))
        cpool = ctx.enter_context(tc.tile_pool(name="cst", bufs=1))
        hpool = ctx.enter_context(tc.tile_pool(name="h", bufs=2))
        spool = ctx.enter_context(tc.tile_pool(name="s", bufs=3))
        gpool = ctx.enter_context(tc.tile_pool(name="g", bufs=2, space="PSUM"))
        tpool = ctx.enter_context(tc.tile_pool(name="t", bufs=2, space="PSUM"))

        whh_t = []
        for k in range(KHT):
            w = wpool.tile([128, NG], BF16, tag=f"whh{k}")
            nc.sync.dma_start(w[:], whh[128 * k : 128 * (k + 1), :])
            whh_t.append(w)
        wih_t = []
        for k in range(KAT):
            w = wpool.tile([128, NG], BF16, tag=f"wih{k}")
            nc.sync.dma_start(w[:], wih[128 * k : 128 * (k + 1), :])
            wih_t.append(w)
        x_t = []
        for k in range(KAT):
            xx = wpool.tile([128, B], BF16, tag=f"x{k}")
            nc.sync.dma_start(xx[:], xT[128 * k : 128 * (k + 1), :])
            x_t.append(xx)
        iden_t = cpool.tile([B, B], BF16, tag="iden")
        nc.sync.dma_start(iden_t[:], iden[:])

        # h^T lives in two half tiles [128, 6*B] (k-tiles 0-5 | 6-11) so the
        # post-AllGather reload is two chunked DMAs and the first matmuls can
        # start as soon as the first chunk lands.
        def load_hT(src):
            halves = []
            for j in range(2):
                hc = hpool.tile([128, 6 * B], BF16, tag=f"hh{j}")
                src_ap = src.rearrange("(k p) n -> p k n", p=128)[
                    :, 6 * j : 6 * j + 6, :
                ]
                nc.sync.dma_start(hc[:], src_ap)
                halves.append(hc)
            return halves

        def h_tile(halves, k):
            return halves[k // 6][:, B * (k % 6) : B * (k % 6 + 1)]

        h_halves = load_hT(h0T)
        c_t = spool.tile([B, HL], F32, tag="c")
        nc.sync.dma_start(c_t[:], c0[:])

        for t in range(S):
            # gates = x^T.T@wih + h^T.T@whh in two 1-bank PSUM groups:
            # g0 = [f|o], g1 = [i|g].  The 9 x-matmuls per group have no h
            # dependency and fill the preceding AllGather window.
            ps = []
            for n in range(2):
                p = gpool.tile([B, NH], F32, tag=f"g{n}")
                nsl = bass.ts(n, NH)
                for k in range(KAT):
                    nc.tensor.matmul(
                        p[:], x_t[k][:], wih_t[k][:, nsl], start=(k == 0), stop=False
                    )
                ps.append(p)
            for n in range(2):
                nsl = bass.ts(n, NH)
                for k in range(KHT):
                    nc.tensor.matmul(
                        ps[n][:],
                        h_tile(h_halves, k),
                        whh_t[k][:, nsl],
                        start=False,
                        stop=(k == KHT - 1),
                    )

            # eltwise: s_fo available while group 1's h-matmuls still stream
            s_fo = spool.tile([B, 2 * HL], F32, tag="sfo")
            nc.scalar.activation(s_fo[:], ps[0][:], sig)
            m1 = spool.tile([B, HL], F32, tag="m1")
            nc.vector.tensor_mul(m1[:], s_fo[:, 0:HL], c_t[:])  # f*c
            s_i = spool.tile([B, HL], F32, tag="si")
            nc.scalar.activation(s_i[:], ps[1][:, 0:HL], sig)
            tg = spool.tile([B, HL], F32, tag="tg")
            nc.scalar.activation(tg[:], ps[1][:, HL : 2 * HL], tanh)
            m2 = spool.tile([B, HL], F32, tag="m2")
            nc.vector.tensor_mul(m2[:], s_i[:], tg[:])  # i*g
            c_new = spool.tile([B, HL], F32, tag="c")
            nc.vector.tensor_add(c_new[:], m1[:], m2[:])
            c_t = c_new
            tc_sb = spool.tile([B, HL], F32, tag="tc")
            nc.scalar.activation(tc_sb[:], c_new[:], tanh)
            h_sb = spool.tile([B, HL], BF16, tag="hsb")
            nc.vector.tensor_mul(h_sb[:], s_fo[:, HL : 2 * HL], tc_sb[:])

            # transpose h [64, 192] -> [192, 64]: two 96-row blocks into one
            # PSUM bank, one copy to SBUF, one strided DMA to the bounce.
            tp = tpool.tile([96, 2 * B], BF16, tag="ht")
            nc.tensor.transpose(tp[:, 0:B], h_sb[:, 0:96], iden_t[:])
            nc.tensor.transpose(tp[:, B : 2 * B], h_sb[:, 96:HL], iden_t[:])
            htc = spool.tile([96, 2 * B], BF16, tag="htc")
            nc.vector.tensor_copy(htc[:], tp[:])
            dst = bounces[t].rearrange("(b p) n -> p b n", p=96)
            src = htc.rearrange("p (b n) -> p b n", b=2)
            nc.sync.dma_start(dst, src)
            # final output written from the bounce copy, off the critical path
            nc.gpsimd.dma_start(out[t, :, :], bounces[t][:])

            if t < S - 1:
                nc.gpsimd.collective_compute(
                    "AllGather",
                    mybir.AluOpType.bypass,
                    replica_groups=[list(range(R))],
                    ins=[bounces[t][:]],
                    outs=[gaths[t][:]],
                )
                h_halves = load_hT(gaths[t])

    nc.compile()
    return nc


def _prep_inputs(sequence, hidden_state, cell_state, W_ih, W_hh, b_ih, b_hh):
    x = np.asarray(sequence, np.float32)[0]          # [64, 1024]
    h0 = np.asarray(hidden_state, np.float32)[0]     # [64, 1536]
    c0f = np.asarray(cell_state, np.float32)[0]
    W_ih = np.asarray(W_ih, np.float32)
    W_hh = np.asarray(W_hh, np.float32)
    b = (np.asarray(b_ih, np.float32) + np.asarray(b_hh, np.float32))

    bf = ml_dtypes.bfloat16
    xT = np.zeros((KA, B), np.float32)
    xT[:IN] = x.T
    xT[IN] = 1.0
    xT = xT.astype(bf)
    h0T = np.ascontiguousarray(h0.T).astype(bf)
    iden = np.eye(B, dtype=bf)

    in_maps = []
    for r in range(R):
        sl = np.arange(r * HL, (r + 1) * HL)
        # per-core gate column order: f, o, i, g
        sel = np.concatenate([H + sl, 3 * H + sl, sl, 2 * H + sl])
        wa = np.zeros((KA, NG), np.float32)
        wa[:IN] = W_ih[sel].T
        wa[IN] = b[sel]
        in_maps.append(
            {
                "xT": xT,
                "wih": wa.astype(bf),
                "whh": np.ascontiguousarray(W_hh[sel].T).astype(bf),
                "h0T": h0T,
                "c0": np.ascontiguousarray(c0f[:, sl]),
                "iden": iden,
            }
        )
    return in_maps


def kernel(**inputs) -> np.ndarray:
    if "nc" not in _CACHE:
        _CACHE["nc"] = _build()
    nc = _CACHE["nc"]
    in_maps = _prep_inputs(**inputs)
    res = bass_utils.run_bass_kernel_spmd(nc, in_maps, core_ids=list(range(R)))
    preds = np.empty((S, B, H), np.float32)
    for r in range(R):
        o = np.asarray(res.results[r]["out"], np.float32)  # [32, 192, 64]
        preds[:, :, r * HL : (r + 1) * HL] = np.transpose(o, (0, 2, 1))
    return preds


# revision 9
# speedup vs baseline: 1.2350x; 1.1479x over previous
"""Trainium2 Bass kernel for nn_Decoder_17214228922493.

32-step LSTM decoder: B=64, H=1536, input=1024, applied to a constant input.
    xg = x @ W_ih.T + b_ih + b_hh                      (once per step, see below)
    per step: gates = xg + h @ W_hh.T ; LSTM cell update ; emit h

Sharding: tensor-parallel over the gate dimension (8 cores x 768 gate
columns); after every step the 8 h^T slices are re-assembled with an
AllGather (mesh, ~6us).  Gate columns are reordered per core to
[f | o | i | g] so one sigmoid covers f,o (whose products with c can start
while the second matmul group is still streaming).

The xg contribution is re-computed from x every step instead of being
injected from a saved tile: those 18 matmuls have no dependency on h, so
the scheduler runs them inside the AllGather window — free work that also
keeps the PE HAM-warm (idle >3.4us would re-throttle the clock 2.4->1.2GHz).

Matmul operands are bf16 (fp32 matmul costs 2 PE passes); PSUM
accumulation and the cell-state arithmetic stay fp32.
"""

import sys

if "/opt/trn_rl_repo" not in sys.path:
    sys.path.insert(0, "/opt/trn_rl_repo")

from contextlib import ExitStack

import ml_dtypes
import numpy as np

import concourse.bass as bass
import concourse.mybir as mybir
import concourse.tile as tile
from concourse import bacc
from concourse import bass_utils
from concourse._compat import get_trn_type

F32 = mybir.dt.float32
BF16 = mybir.dt.bfloat16
R = 8          # cores
B = 64         # batch
H = 1536       # hidden
HL = H // R    # 192 per-core hidden slice
IN = 1024      # lstm input size
KA = 1152      # augmented input contraction (1024 + bias row, padded to 9*128)
NG = 4 * HL    # 768 gate columns per core
S = 32         # steps
NH = 384       # matmul moving free-dim (two groups of 384 = NG)
KHT = H // 128   # 12 k-tiles for the recurrent matmul
KAT = KA // 128  # 9 k-tiles for the input matmul

_CACHE = {}


def _build():
    nc = bacc.Bacc(
        get_trn_type() or "TRN2",
        target_bir_lowering=False,
        debug=False,
        num_devices=R,
    )

    xT = nc.dram_tensor("xT", [KA, B], BF16, kind="ExternalInput")
    wih = nc.dram_tensor("wih", [KA, NG], BF16, kind="ExternalInput")
    whh = nc.dram_tensor("whh", [H, NG], BF16, kind="ExternalInput")
    h0T = nc.dram_tensor("h0T", [H, B], BF16, kind="ExternalInput")
    c0 = nc.dram_tensor("c0", [B, HL], F32, kind="ExternalInput")
    iden = nc.dram_tensor("iden", [B, B], BF16, kind="ExternalInput")
    out = nc.dram_tensor("out", [S, HL, B], BF16, kind="ExternalOutput")

    bounces = [
        nc.dram_tensor(f"bounce{t}", [HL, B], BF16, kind="Internal") for t in range(S)
    ]
    gaths = [
        nc.dram_tensor(f"gath{t}", [H, B], BF16, kind="Internal", addr_space="Shared")
        for t in range(S - 1)
    ]

    sig = mybir.ActivationFunctionType.Sigmoid
    tanh = mybir.ActivationFunctionType.Tanh

    with ExitStack() as ctx:
        tc = ctx.enter_context(tile.TileContext(nc))
        wpool = ctx.enter_context(tc.tile_pool(name="w", bufs=1))
        cpool = ctx.enter_context(tc.tile_pool(name="cst", bufs=1))
        hpool = ctx.enter_context(tc.tile_pool(name="h", bufs=2))
        spool = ctx.enter_context(tc.tile_pool(name="s", bufs=3))
        gpool = ctx.enter_context(tc.tile_pool(name="g", bufs=2, space="PSUM"))
        tpool = ctx.enter_context(tc.tile_pool(name="t", bufs=2, space="PSUM"))

        whh_t = []
        for k in range(KHT):
            w = wpool.tile([128, NG], BF16, tag=f"whh{k}")
            nc.sync.dma_start(w[:], whh[128 * k : 128 * (k + 1), :])
            whh_t.append(w)
        wih_t = []
        for k in range(KAT):
            w = wpool.tile([128, NG], BF16, tag=f"wih{k}")
            nc.sync.dma_start(w[:], wih[128 * k : 128 * (k + 1), :])
            wih_t.append(w)
        x_t = []
        for k in range(KAT):
            xx = wpool.tile([128, B], BF16, tag=f"x{k}")
            nc.sync.dma_start(xx[:], xT[128 * k : 128 * (k + 1), :])
            x_t.append(xx)
        iden_t = cpool.tile([B, B], BF16, tag="iden")
        nc.sync.dma_start(iden_t[:], iden[:])

        # h^T lives in three tiles [128, 4*B] (k-tiles 0-3 | 4-7 | 8-11) so the
        # post-AllGather reload is three chunked DMAs and the first matmuls can
        # start as soon as the first chunk lands.
        def load_hT(src):
            halves = []
            for j in range(3):
                hc = hpool.tile([128, 4 * B], BF16, tag=f"hh{j}")
                src_ap = src.rearrange("(k p) n -> p k n", p=128)[
                    :, 4 * j : 4 * j + 4, :
                ]
                nc.sync.dma_start(hc[:], src_ap)
                halves.append(hc)
            return halves

        def h_tile(halves, k):
            return halves[k // 4][:, B * (k % 4) : B * (k % 4 + 1)]

        h_halves = load_hT(h0T)
        c_t = spool.tile([B, HL], F32, tag="c")
        nc.sync.dma_start(c_t[:], c0[:])

        prev_bounce_dma = None
        for t in range(S):
            # gates = x^T.T@wih + h^T.T@whh in two 1-bank PSUM groups:
            # g0 = [f|o], g1 = [i|g].  The 9 x-matmuls per group have no h
            # dependency and fill the preceding AllGather window.
            ps = []
            for n in range(2):
                p = gpool.tile([B, NH], F32, tag=f"g{n}")
                nsl = bass.ts(n, NH)
                for k in range(KAT):
                    mm = nc.tensor.matmul(
                        p[:], x_t[k][:], wih_t[k][:, nsl], start=(k == 0), stop=False
                    )
                    if n == 0 and k == 0 and prev_bounce_dma is not None:
                        # hold the x-matmuls until the h slice has shipped, so
                        # they run inside the AllGather window (keeps the PE
                        # HAM-warm through the gather instead of before it)
                        tile.add_dep_helper(
                            mm.ins,
                            prev_bounce_dma.ins,
                            sync=True,
                            reason="x-matmuls fill the AllGather window",
                        )
                ps.append(p)
            for n in range(2):
                nsl = bass.ts(n, NH)
                for k in range(KHT):
                    nc.tensor.matmul(
                        ps[n][:],
                        h_tile(h_halves, k),
                        whh_t[k][:, nsl],
                        start=False,
                        stop=(k == KHT - 1),
                    )

            # eltwise: s_fo available while group 1's h-matmuls still stream
            s_fo = spool.tile([B, 2 * HL], F32, tag="sfo")
            nc.scalar.activation(s_fo[:], ps[0][:], sig)
            m1 = spool.tile([B, HL], F32, tag="m1")
            nc.vector.tensor_mul(m1[:], s_fo[:, 0:HL], c_t[:])  # f*c
            s_i = spool.tile([B, HL], F32, tag="si")
            nc.scalar.activation(s_i[:], ps[1][:, 0:HL], sig)
            tg = spool.tile([B, HL], F32, tag="tg")
            nc.scalar.activation(tg[:], ps[1][:, HL : 2 * HL], tanh)
            m2 = spool.tile([B, HL], F32, tag="m2")
            nc.vector.tensor_mul(m2[:], s_i[:], tg[:])  # i*g
            c_new = spool.tile([B, HL], F32, tag="c")
            nc.vector.tensor_add(c_new[:], m1[:], m2[:])
            c_t = c_new
            tc_sb = spool.tile([B, HL], F32, tag="tc")
            nc.scalar.activation(tc_sb[:], c_new[:], tanh)
            h_sb = spool.tile([B, HL], BF16, tag="hsb")
            nc.vector.tensor_mul(h_sb[:], s_fo[:, HL : 2 * HL], tc_sb[:])

            # transpose h [64, 192] -> [192, 64]: two 96-row blocks into one
            # PSUM bank, one copy to SBUF, one strided DMA to the bounce.
            tp = tpool.tile([96, 2 * B], BF16, tag="ht")
            nc.tensor.transpose(tp[:, 0:B], h_sb[:, 0:96], iden_t[:])
            nc.tensor.transpose(tp[:, B : 2 * B], h_sb[:, 96:HL], iden_t[:])
            htc = spool.tile([96, 2 * B], BF16, tag="htc")
            nc.vector.tensor_copy(htc[:], tp[:])
            dst = bounces[t].rearrange("(b p) n -> p b n", p=96)
            src = htc.rearrange("p (b n) -> p b n", b=2)
            prev_bounce_dma = nc.sync.dma_start(dst, src)

            if t < S - 1:
                nc.gpsimd.collective_compute(
                    "AllGather",
                    mybir.AluOpType.bypass,
                    replica_groups=[list(range(R))],
                    ins=[bounces[t][:]],
                    outs=[gaths[t][:]],
                )
                h_halves = load_hT(gaths[t])
            # final output written from the bounce copy, off the critical path
            # (after the collective trigger so it never delays it)
            nc.gpsimd.dma_start(out[t, :, :], bounces[t][:])

    nc.compile()
    return nc


def _prep_inputs(sequence, hidden_state, cell_state, W_ih, W_hh, b_ih, b_hh):
    x = np.asarray(sequence, np.float32)[0]          # [64, 1024]
    h0 = np.asarray(hidden_state, np.float32)[0]     # [64, 1536]
    c0f = np.asarray(cell_state, np.float32)[0]
    W_ih = np.asarray(W_ih, np.float32)
    W_hh = np.asarray(W_hh, np.float32)
    b = (np.asarray(b_ih, np.float32) + np.asarray(b_hh, np.float32))

    bf = ml_dtypes.bfloat16
    xT = np.zeros((KA, B), np.float32)
    xT[:IN] = x.T
    xT[IN] = 1.0
    xT = xT.astype(bf)
    h0T = np.ascontiguousarray(h0.T).astype(bf)
    iden = np.eye(B, dtype=bf)

    in_maps = []
    for r in range(R):
        sl = np.arange(r * HL, (r + 1) * HL)
        # per-core gate column order: f, o, i, g
        sel = np.concatenate([H + sl, 3 * H + sl, sl, 2 * H + sl])
        wa = np.zeros((KA, NG), np.float32)
        wa[:IN] = W_ih[sel].T
        wa[IN] = b[sel]
        in_maps.append(
            {
                "xT": xT,
                "wih": wa.astype(bf),
                "whh": np.ascontiguousarray(W_hh[sel].T).astype(bf),
                "h0T": h0T,
                "c0": np.ascontiguousarray(c0f[:, sl]),
                "iden": iden,
            }
        )
    return in_maps


def kernel(**inputs) -> np.ndarray:
    if "nc" not in _CACHE:
        _CACHE["nc"] = _build()
    nc = _CACHE["nc"]
    in_maps = _prep_inputs(**inputs)
    res = bass_utils.run_bass_kernel_spmd(nc, in_maps, core_ids=list(range(R)))
    preds = np.empty((S, B, H), np.float32)
    for r in range(R):
        o = np.asarray(res.results[r]["out"], np.float32)  # [32, 192, 64]
        preds[:, :, r * HL : (r + 1) * HL] = np.transpose(o, (0, 2, 1))
    return preds


# revision 11
# speedup vs baseline: 1.2677x; 1.0265x over previous
"""Trainium2 Bass kernel for nn_Decoder_17214228922493.

32-step LSTM decoder: B=64, H=1536, input=1024, applied to a constant input.
    xg = x @ W_ih.T + b_ih + b_hh                      (once per step, see below)
    per step: gates = xg + h @ W_hh.T ; LSTM cell update ; emit h

Sharding: tensor-parallel over the gate dimension (8 cores x 768 gate
columns); after every step the 8 h^T slices are re-assembled with an
AllGather (mesh, ~6us).  Gate columns are reordered per core to
[f | o | i | g] so one sigmoid covers f,o (whose products with c can start
while the second matmul group is still streaming).

The xg contribution is re-computed from x every step instead of being
injected from a saved tile: those 18 matmuls have no dependency on h, so
the scheduler runs them inside the AllGather window — free work that also
keeps the PE HAM-warm (idle >3.4us would re-throttle the clock 2.4->1.2GHz).

Matmul operands are bf16 (fp32 matmul costs 2 PE passes); PSUM
accumulation and the cell-state arithmetic stay fp32.
"""

import sys

if "/opt/trn_rl_repo" not in sys.path:
    sys.path.insert(0, "/opt/trn_rl_repo")

from contextlib import ExitStack

import ml_dtypes
import numpy as np

import concourse.bass as bass
import concourse.mybir as mybir
import concourse.tile as tile
from concourse import bacc
from concourse import bass_utils
from concourse._compat import get_trn_type

F32 = mybir.dt.float32
BF16 = mybir.dt.bfloat16
R = 8          # cores
B = 64         # batch
H = 1536       # hidden
HL = H // R    # 192 per-core hidden slice
IN = 1024      # lstm input size
KA = 1152      # augmented input contraction (1024 + bias row, padded to 9*128)
NG = 4 * HL    # 768 gate columns per core
S = 32         # steps
NH = 384       # matmul moving free-dim (two groups of 384 = NG)
KHT = H // 128   # 12 k-tiles for the recurrent matmul
KAT = KA // 128  # 9 k-tiles for the input matmul

_CACHE = {}


def _build():
    nc = bacc.Bacc(
        get_trn_type() or "TRN2",
        target_bir_lowering=False,
        debug=False,
        num_devices=R,
    )

    xT = nc.dram_tensor("xT", [KA, B], BF16, kind="ExternalInput")
    wih = nc.dram_tensor("wih", [KA, NG], BF16, kind="ExternalInput")
    whh = nc.dram_tensor("whh", [H, NG], BF16, kind="ExternalInput")
    h0T = nc.dram_tensor("h0T", [H, B], BF16, kind="ExternalInput")
    c0 = nc.dram_tensor("c0", [B, HL], F32, kind="ExternalInput")
    iden = nc.dram_tensor("iden", [B, B], BF16, kind="ExternalInput")
    out = nc.dram_tensor("out", [S, HL, B], BF16, kind="ExternalOutput")

    bounces = [
        nc.dram_tensor(f"bounce{t}", [HL, B], BF16, kind="Internal") for t in range(S)
    ]
    gaths = [
        nc.dram_tensor(f"gath{t}", [H, B], BF16, kind="Internal", addr_space="Shared")
        for t in range(S - 1)
    ]

    sig = mybir.ActivationFunctionType.Sigmoid
    tanh = mybir.ActivationFunctionType.Tanh

    with ExitStack() as ctx:
        tc = ctx.enter_context(tile.TileContext(nc))
        wpool = ctx.enter_context(tc.tile_pool(name="w", bufs=1))
        cpool = ctx.enter_context(tc.tile_pool(name="cst", bufs=1))
        hpool = ctx.enter_context(tc.tile_pool(name="h", bufs=2))
        spool = ctx.enter_context(tc.tile_pool(name="s", bufs=3))
        gpool = ctx.enter_context(tc.tile_pool(name="g", bufs=2, space="PSUM"))
        tpool = ctx.enter_context(tc.tile_pool(name="t", bufs=2, space="PSUM"))

        whh_t = []
        for k in range(KHT):
            w = wpool.tile([128, NG], BF16, tag=f"whh{k}")
            nc.sync.dma_start(w[:], whh[128 * k : 128 * (k + 1), :])
            whh_t.append(w)
        wih_t = []
        for k in range(KAT):
            w = wpool.tile([128, NG], BF16, tag=f"wih{k}")
            nc.sync.dma_start(w[:], wih[128 * k : 128 * (k + 1), :])
            wih_t.append(w)
        x_t = []
        for k in range(KAT):
            xx = wpool.tile([128, B], BF16, tag=f"x{k}")
            nc.sync.dma_start(xx[:], xT[128 * k : 128 * (k + 1), :])
            x_t.append(xx)
        iden_t = cpool.tile([B, B], BF16, tag="iden")
        nc.sync.dma_start(iden_t[:], iden[:])

        # h^T lives in three tiles [128, 4*B] (k-tiles 0-3 | 4-7 | 8-11) so the
        # post-AllGather reload is three chunked DMAs and the first matmuls can
        # start as soon as the first chunk lands.
        reload_engines = [nc.sync, nc.scalar, nc.gpsimd]

        def load_hT(src):
            halves = []
            for j in range(3):
                hc = hpool.tile([128, 4 * B], BF16, tag=f"hh{j}")
                src_ap = src.rearrange("(k p) n -> p k n", p=128)[
                    :, 4 * j : 4 * j + 4, :
                ]
                reload_engines[j].dma_start(hc[:], src_ap)
                halves.append(hc)
            return halves

        def h_tile(halves, k):
            return halves[k // 4][:, B * (k % 4) : B * (k % 4 + 1)]

        h_halves = load_hT(h0T)
        c_t = spool.tile([B, HL], F32, tag="c")
        nc.sync.dma_start(c_t[:], c0[:])

        prev_bounce_dma = None
        for t in range(S):
            # gates = x^T.T@wih + h^T.T@whh in two 1-bank PSUM groups:
            # g0 = [f|o], g1 = [i|g].  The 9 x-matmuls per group have no h
            # dependency and fill the preceding AllGather window.
            ps = []
            for n in range(2):
                p = gpool.tile([B, NH], F32, tag=f"g{n}")
                nsl = bass.ts(n, NH)
                for k in range(KAT):
                    mm = nc.tensor.matmul(
                        p[:], x_t[k][:], wih_t[k][:, nsl], start=(k == 0), stop=False
                    )
                    if n == 0 and k == 0 and prev_bounce_dma is not None:
                        # hold the x-matmuls until the h slice has shipped, so
                        # they run inside the AllGather window (keeps the PE
                        # HAM-warm through the gather instead of before it)
                        tile.add_dep_helper(
                            mm.ins,
                            prev_bounce_dma.ins,
                            sync=True,
                            reason="x-matmuls fill the AllGather window",
                        )
                        # paced PE<->DVE ping-pong: one tiny matmul every
                        # ~0.8us of semaphore round-trip keeps the PE's HAM
                        # activity window non-idle across the AllGather tail
                        # and the reload, so the h-matmuls run at 2.4GHz.
                        hb_prev = None
                        for hb_i in range(7):
                            hb_ps = gpool.tile([B, B], F32, tag="hb")
                            hb_mm = nc.tensor.matmul(
                                hb_ps[:],
                                iden_t[:],
                                iden_t[:] if hb_prev is None else hb_prev[:],
                                start=True,
                                stop=True,
                            )
                            if hb_i == 0:
                                tile.add_dep_helper(
                                    hb_mm.ins,
                                    prev_bounce_dma.ins,
                                    sync=True,
                                    reason="heartbeat anchored to AllGather window",
                                )
                            hb_sb = spool.tile([B, B], BF16, tag="hbs")
                            nc.vector.tensor_copy(hb_sb[:], hb_ps[:])
                            hb_prev = hb_sb
                ps.append(p)
            for n in range(2):
                nsl = bass.ts(n, NH)
                for k in range(KHT):
                    nc.tensor.matmul(
                        ps[n][:],
                        h_tile(h_halves, k),
                        whh_t[k][:, nsl],
                        start=False,
                        stop=(k == KHT - 1),
                    )

            # eltwise: group 0 = [g|f] finishes first -> tanh(g), sigmoid(f)
            # and f*c all run while group 1's h-matmuls still stream; group 1
            # = [o|i] needs a single wide sigmoid.
            tg = spool.tile([B, HL], F32, tag="tg")
            nc.scalar.activation(tg[:], ps[0][:, 0:HL], tanh)
            s_f = spool.tile([B, HL], F32, tag="sf")
            nc.scalar.activation(s_f[:], ps[0][:, HL : 2 * HL], sig)
            m1 = spool.tile([B, HL], F32, tag="m1")
            nc.vector.tensor_mul(m1[:], s_f[:], c_t[:])  # f*c
            s_oi = spool.tile([B, 2 * HL], F32, tag="soi")
            nc.scalar.activation(s_oi[:], ps[1][:], sig)
            m2 = spool.tile([B, HL], F32, tag="m2")
            nc.vector.tensor_mul(m2[:], s_oi[:, HL : 2 * HL], tg[:])  # i*g
            c_new = spool.tile([B, HL], F32, tag="c")
            nc.vector.tensor_add(c_new[:], m1[:], m2[:])
            c_t = c_new
            tc_sb = spool.tile([B, HL], F32, tag="tc")
            nc.scalar.activation(tc_sb[:], c_new[:], tanh)
            h_sb = spool.tile([B, HL], BF16, tag="hsb")
            nc.vector.tensor_mul(h_sb[:], s_oi[:, 0:HL], tc_sb[:])

            # transpose h [64, 192] -> [192, 64]: two 96-row blocks into one
            # PSUM bank, one copy to SBUF, one strided DMA to the bounce.
            tp = tpool.tile([96, 2 * B], BF16, tag="ht")
            nc.tensor.transpose(tp[:, 0:B], h_sb[:, 0:96], iden_t[:])
            nc.tensor.transpose(tp[:, B : 2 * B], h_sb[:, 96:HL], iden_t[:])
            htc = spool.tile([96, 2 * B], BF16, tag="htc")
            nc.vector.tensor_copy(htc[:], tp[:])
            dst = bounces[t].rearrange("(b p) n -> p b n", p=96)
            src = htc.rearrange("p (b n) -> p b n", b=2)
            prev_bounce_dma = nc.sync.dma_start(dst, src)

            if t < S - 1:
                nc.gpsimd.collective_compute(
                    "AllGather",
                    mybir.AluOpType.bypass,
                    replica_groups=[list(range(R))],
                    ins=[bounces[t][:]],
                    outs=[gaths[t][:]],
                )
                h_halves = load_hT(gaths[t])
            # final output written from the bounce copy, off the critical path
            # (after the collective trigger so it never delays it)
            nc.gpsimd.dma_start(out[t, :, :], bounces[t][:])

    nc.compile()
    return nc


def _prep_inputs(sequence, hidden_state, cell_state, W_ih, W_hh, b_ih, b_hh):
    x = np.asarray(sequence, np.float32)[0]          # [64, 1024]
    h0 = np.asarray(hidden_state, np.float32)[0]     # [64, 1536]
    c0f = np.asarray(cell_state, np.float32)[0]
    W_ih = np.asarray(W_ih, np.float32)
    W_hh = np.asarray(W_hh, np.float32)
    b = (np.asarray(b_ih, np.float32) + np.asarray(b_hh, np.float32))

    bf = ml_dtypes.bfloat16
    xT = np.zeros((KA, B), np.float32)
    xT[:IN] = x.T
    xT[IN] = 1.0
    xT = xT.astype(bf)
    h0T = np.ascontiguousarray(h0.T).astype(bf)
    iden = np.eye(B, dtype=bf)

    in_maps = []
    for r in range(R):
        sl = np.arange(r * HL, (r + 1) * HL)
        # per-core gate column order: g, f, o, i
        sel = np.concatenate([2 * H + sl, H + sl, 3 * H + sl, sl])
        wa = np.zeros((KA, NG), np.float32)
        wa[:IN] = W_ih[sel].T
        wa[IN] = b[sel]
        in_maps.append(
            {
                "xT": xT,
                "wih": wa.astype(bf),
                "whh": np.ascontiguousarray(W_hh[sel].T).astype(bf),
                "h0T": h0T,
                "c0": np.ascontiguousarray(c0f[:, sl]),
                "iden": iden,
            }
        )
    return in_maps


def kernel(**inputs) -> np.ndarray:
    if "nc" not in _CACHE:
        _CACHE["nc"] = _build()
    nc = _CACHE["nc"]
    in_maps = _prep_inputs(**inputs)
    res = bass_utils.run_bass_kernel_spmd(nc, in_maps, core_ids=list(range(R)))
    preds = np.empty((S, B, H), np.float32)
    for r in range(R):
        o = np.asarray(res.results[r]["out"], np.float32)  # [32, 192, 64]
        preds[:, :, r * HL : (r + 1) * HL] = np.transpose(o, (0, 2, 1))
    return preds
